# revision 75
# baseline (speedup 1.0000x reference)
"""Binarized 3x3 conv (BinarizeConv2dSDP) for one TRN2 chip (8 NeuronCores).

Reference computation:
    out = conv2d(sign(x), sign(M), stride=1, pad=1) * Alpha      (all fp32)
    x: (32, 256, 56, 56)   M: (256, 256, 3, 3)   Alpha: (256, 1, 1)

Strategy (per the data-parallel sharding hint):
  - Shard x over batch: 4 images per core; replicate the weights/Alpha on
    every core.
  - The weight tensor (0.004% of the FLOPs) is packed on the host into the
    fp8 +/-1 transposed-lhsT layout the PE consumes (sign() is exact in
    fp8, so this is pure input marshalling); each core DMAs the 0.59MB
    packed block once. The conv itself - all 59 GFLOP - runs on device.
  - On-core: binarize x to fp8 on the ACT engine, run the conv as 9
    shifted DoubleRow matmuls per 8-row strip (contraction = 256 channels:
    128 partitions x 2 pair-rows) accumulating in PSUM, scale by Alpha
    while evacuating PSUM, DMA out fp32.
  - Activations live in SBUF as zero-padded 58x58 images. Each matmul's
    moving AP is 2-level [8 rows x 56 cols] (row stride 58), so only the
    448 valid output columns of a strip are computed - no seam garbage.
  - Startup is latency-tuned: the packed weights land first on the sync
    HWDGE queue, image-0 arrives as 9/26/23-row chunks split across the
    sync+scalar queues, image-0's strips alternate ot blocks so early conv
    consumption matches the DMA arrival rate, act-tile zero fills run on
    GpSimd, and a short warmup burst ramps the HAM clock gate. Later
    image DMAs dispatch from conv-loop hooks (dispatching early would
    exhaust HWDGE ring credit and block the ACT engine's sign stream).
"""

import time

import numpy as np

import concourse.bacc as bacc
import concourse.bass as bass
import concourse.tile as tile
from concourse import mybir
from concourse.bass_utils import run_bass_kernel_spmd

F32 = mybir.dt.float32
BF16 = mybir.dt.bfloat16
FP8 = mybir.dt.float8e4
U8 = mybir.dt.uint8

# ---- problem geometry (hardcoded; kernel.py must be self-contained) ----
N_CORES = 8
NB = 4          # images per core (32 / 8)
C = 256         # in channels  (2 halves of 128 partitions)
O = 256         # out channels (2 tiles of 128 partitions)
H = W = 56
K = 3
PW = H + 2      # padded row width  (58)
NPIX = PW * PW  # padded image size (3364)
PH = 3376       # padded image allocation (multiple of 16)
RS = 8          # output rows per strip
NSTRIP = H // RS        # 7
NVAL = RS * W           # 448 psum columns per strip (valid only)

# image-0 chunking (input-row ranges) for early conv start
U_R0, U_NR = 0, 9       # ultra chunk -> early tile, strip 0
B_R0, B_NR = 7, 26      # covers padded rows 8..33 (strips 1-3, + s4 top)
C_R0, C_NR = 33, 23     # covers padded rows 34..56 (strips 4-6)
EROWS = U_NR + 1        # early tile padded rows 0..9
EPH = 592               # 10*58=580 rounded up to a multiple of 16

N_WARM = 62             # warmup matmuls (HAM clock ramp, fills PE to conv)
OUT_SPLIT = 12          # out-DMAs before this gidx go via GpSimd SWDGE


def build_nc() -> bass.Bass:
    """Build the SPMD Bass program for one core's shard."""
    nc = bacc.Bacc("TRN2")

    x = nc.declare_dram_parameter("x", [NB, C, H, W], F32, isOutput=False)
    # host-packed weights: wq[c2, half, kk*256 + ot*128 + o2]
    #   = sign(M[ot*128+o2, half*128+c2, kh, kw]) as fp8e4 bytes
    wq = nc.declare_dram_parameter("wq", [128, 2, K * K * O], U8, isOutput=False)
    alpha = nc.declare_dram_parameter("alpha", [O], F32, isOutput=False)
    out = nc.declare_dram_parameter("out", [NB, O, H, W], F32, isOutput=True)

    with tile.TileContext(nc) as tc:
        with (
            tc.tile_pool(name="consts", bufs=1) as consts,
            tc.tile_pool(name="xsrc", bufs=6) as xsrc_pool,
            tc.tile_pool(name="xsrc0", bufs=6) as xsrc0_pool,
            tc.tile_pool(name="osb", bufs=8) as osb_pool,
            tc.tile_pool(name="pmm", bufs=6, space="PSUM") as pmm_pool,
        ):
            # alpha: two tiny scattered DMAs; ot=0 warms the GpSimd SWDGE
            # ring, ot=1 warms the scalar HWDGE queue ahead of wq-h1 (the
            # cold scalar queue otherwise crawls for ~9us on its first MB)
            alpha_sb = consts.tile([128, 2], F32)
            for ot, eng in ((0, nc.gpsimd), (1, nc.scalar)):
                eng.dma_start(
                    out=alpha_sb[:, ot : ot + 1],
                    in_=alpha.rearrange("(t o) -> t o", t=2)[ot].rearrange(
                        "(o u) -> o u", u=1
                    ),
                )

            # wz before any DVE dma dispatch: warmup matmuls gate on it
            wz = consts.tile([128, 256], BF16)
            nc.vector.memset(wz[:], 0)

            wbuf = consts.tile([128, 2, K * K * O], FP8)

            def x_chunk_dma(eng, n, half, r0, nr):
                xs = xsrc0_pool.tile([128, B_NR * W], F32)
                eng.dma_start(
                    out=xs[:, : nr * W],
                    in_=x[n, half * 128 : (half + 1) * 128, r0 : r0 + nr, :].rearrange(
                        "c h w -> c (h w)"
                    ),
                )
                return xs

            def x_img_dma(eng, n, half):
                xs = xsrc_pool.tile([128, H * W], F32)
                eng.dma_start(
                    out=xs[:],
                    in_=x[n, half * 128 : (half + 1) * 128].rearrange(
                        "c h w -> c (h w)"
                    ),
                )
                return xs

            # conv-start-critical chunks (xu, wq) ride the fast sync queue.
            # xu goes FIRST: it needs an ACT sign after landing while the
            # packed weights need no post-processing, so the last-arriving
            # transfer is the one without a processing tail. B/C go to the
            # scalar queue, which gets the DMA engines to itself once sync
            # quiets (~16us).
            # both xu halves in ONE dispatch: wq-h0's dispatch (and queue
            # entry) moves ~0.7us earlier on the sync engine
            xuc = xsrc0_pool.tile([128, 2, U_NR * W], F32)
            nc.sync.dma_start(
                out=xuc[:],
                in_=x[0, :, U_R0 : U_R0 + U_NR, :].rearrange(
                    "(a c) h w -> c a (h w)", a=2
                ),
            )
            xu = [xuc[:, h2] for h2 in range(2)]
            # wq split by c-half across both queues: halves the descriptor
            # rounds behind xu and overlaps the two transfers. On scalar,
            # wq-h1 is sandwiched between the B halves: B0 first for the
            # strip-1 deadline, wq-h1 next for an early conv start.
            nc.sync.dma_start(out=wbuf[:, 0].bitcast(U8), in_=wq[:, 0])
            nc.scalar.dma_start(out=wbuf[:, 1].bitcast(U8), in_=wq[:, 1])
            xb = [x_chunk_dma(nc.scalar, 0, h2, B_R0, B_NR) for h2 in range(2)]
            xc = [x_chunk_dma(nc.sync, 0, h2, C_R0, C_NR) for h2 in range(2)]

            # ---- PE warm-up: ramps the HAM clock gate (K=4/8 -> 8/8)
            # while the weight + image-0 DMAs land ----
            # bulk warmups bridge the PE to the typical conv-ready time
            # (~17.5us); a <=3.2us idle gap re-throttles HAM to half clock
            # for 10us, which costs far more than a slight overshoot. The
            # small trailing matmuls give the bridge fine granularity.
            pwarm = pmm_pool.tile([128, NVAL], F32, tag="pm")
            for _ in range(N_WARM):
                nc.tensor.matmul(
                    pwarm[:, :256], wz[:, :128], wz[:], start=True, stop=True
                )
            for _ in range(6):
                nc.tensor.matmul(
                    pwarm[:, :64], wz[:, :128], wz[:, :64], start=True, stop=True
                )

            # activation tiles: zero fill on GpSimd (keeps DVE/ACT free).
            # Image 0 + early tile + image 1 now; images 2-3 deferred.
            act = consts.tile([128, 2 * NB, PH], FP8)
            acte = consts.tile([128, 2, EPH], FP8)
            nc.gpsimd.memset(
                acte.rearrange("p a b -> p (a b)").bitcast(mybir.dt.uint32), 0
            )

            def act_memset(n):
                nc.gpsimd.memset(
                    act[:, 2 * n : 2 * n + 2, :]
                    .rearrange("p a b -> p (a b)")
                    .bitcast(mybir.dt.uint32),
                    0,
                )

            act_memset(0)
            act_memset(1)

            # ---- image-0 signs: ultra -> early tile, B/C -> main act ----
            for h2 in range(2):
                dst = acte[:, h2, : EROWS * PW].rearrange("p (h w) -> p h w", w=PW)[
                    :, 1 : 1 + U_NR, 1 : W + 1
                ]
                nc.scalar.sign(
                    dst, xu[h2][:, : U_NR * W].rearrange("p (h w) -> p h w", w=W)
                )

            # B in two row chunks, in DMA-arrival order, so strips 1-3
            # unblock as early as possible
            def b_sign(h2, r0, r1):
                dst = act[:, h2, :NPIX].rearrange("p (h w) -> p h w", w=PW)[
                    :, 1 + B_R0 + r0 : 1 + B_R0 + r1, 1 : W + 1
                ]
                nc.scalar.sign(
                    dst,
                    xb[h2][:, r0 * W : r1 * W].rearrange("p (h w) -> p h w", w=W),
                )

            b_sign(0, 0, 10)
            b_sign(1, 0, 10)
            b_sign(0, 10, B_NR)
            b_sign(1, 10, B_NR)
            for h2 in range(2):
                dst = act[:, h2, :NPIX].rearrange("p (h w) -> p h w", w=PW)[
                    :, 1 + C_R0 : 1 + C_R0 + C_NR, 1 : W + 1
                ]
                nc.scalar.sign(
                    dst, xc[h2][:, : C_NR * W].rearrange("p (h w) -> p h w", w=W)
                )

            def sign_img(n, half, xs):
                dst = act[:, 2 * n + half, :NPIX].rearrange(
                    "p (h w) -> p h w", w=PW
                )[:, 1 : H + 1, 1 : W + 1]
                nc.scalar.sign(dst, xs.rearrange("p (h w) -> p h w", w=W))

            # deferred work hooks: (gidx -> thunks) slipped into the conv
            # loop. Image DMAs dispatch here (ring credit is free by then);
            # their signs follow once the transfers have landed.
            ximg = {}

            def img_dma(eng, n, half):
                ximg[(n, half)] = x_img_dma(eng, n, half)

            def img_sign(n, half):
                sign_img(n, half, ximg[(n, half)])

            # image 1 follows image-0's chunks on the sync queue; images
            # 2-3 split across sync+scalar with wide deadline margins
            hooks = {
                4: [lambda: img_dma(nc.sync, 1, 0)],
                6: [lambda: img_dma(nc.sync, 1, 1)],
                8: [lambda: img_sign(1, 0), lambda: act_memset(2)],
                10: [lambda: img_sign(1, 1), lambda: img_dma(nc.sync, 2, 0)],
                12: [lambda: img_dma(nc.scalar, 2, 1)],
                14: [lambda: act_memset(3), lambda: img_dma(nc.sync, 3, 0)],
                16: [lambda: img_sign(2, 0), lambda: img_dma(nc.scalar, 3, 1)],
                17: [lambda: img_sign(2, 1)],
                20: [lambda: img_sign(3, 0)],
                21: [lambda: img_sign(3, 1)],
            }

            # ---- main conv loop ----
            def conv_strip(n, ot, s, gidx, r0=0, nr=RS, last_piece=False):
                pm = pmm_pool.tile([128, nr * W], F32)
                for kk in range(K * K):
                    kh, kw = divmod(kk, K)
                    base = (RS * s + r0 + kh) * PW + kw
                    lhsT2 = wbuf[:, :, kk * O + ot * 128 : kk * O + ot * 128 + 128]
                    src = acte if (n == 0 and s == 0) else act[:, 2 * n : 2 * n + 2]
                    # 2-level moving AP: nr rows x 56 valid cols
                    rhs4 = src[:, :, base : base + nr * PW].rearrange(
                        "p a (r w) -> p a r w", w=PW
                    )[:, :, :, :W]
                    nc.tensor.matmul(
                        pm[:],
                        lhsT2,
                        rhs4,
                        start=(kk == 0),
                        stop=(kk == K * K - 1),
                        perf_mode=mybir.MatmulPerfMode.DoubleRow,
                    )
                # evacuate, scaled by per-channel alpha
                osb = osb_pool.tile([128, nr * W], F32)
                nc.vector.tensor_scalar_mul(osb[:], pm[:], alpha_sb[:, ot : ot + 1])
                # early outs via GpSimd SWDGE (sync HWDGE is busy with
                # inputs); late outs via the then-idle sync ring; the final
                # strips' halves drain on parallel queues (only sync,
                # scalar and gpsimd can dispatch DMAs)
                if gidx < OUT_SPLIT:
                    eng = nc.gpsimd
                elif r0 > 0:
                    eng = nc.scalar
                else:
                    eng = nc.sync
                eng.dma_start(
                    out=out[
                        n,
                        ot * 128 : (ot + 1) * 128,
                        RS * s + r0 : RS * s + r0 + nr,
                        :,
                    ].rearrange("o h w -> o (h w)"),
                    in_=osb[:],
                )

            # image 0 alternates ot within each strip so early conv
            # consumption (2 groups per strip) tracks the chunk arrivals;
            # later images go ot-major (their data is fully resident).
            order = [(0, ot, s) for s in range(NSTRIP) for ot in range(2)]
            for n in range(1, NB):
                order += [(n, ot, s) for ot in range(2) for s in range(NSTRIP)]

            for gidx, (n, ot, s) in enumerate(order):
                if gidx == len(order) - 1:
                    # final strip in shrinking pieces (4,2,1,1 rows): each
                    # piece's evac+DMA overlaps the next piece's matmuls,
                    # and the very last transfer carries only one row
                    for r0, nr in ((0, 4), (4, 2), (6, 1), (7, 1)):
                        conv_strip(n, ot, s, gidx, r0, nr)
                elif gidx == len(order) - 2:
                    conv_strip(n, ot, s, gidx, 0, RS // 2)
                    conv_strip(n, ot, s, gidx, RS // 2, RS // 2)
                else:
                    conv_strip(n, ot, s, gidx)
                for h in hooks.get(gidx, ()):
                    h()
    nc.finalize()
    return nc


_NC_CACHE: dict[bool, bass.Bass] = {}


def get_nc(paired: bool = True) -> bass.Bass:
    if paired not in _NC_CACHE:
        _NC_CACHE[paired] = build_nc()
    return _NC_CACHE[paired]


def pack_weights(M: np.ndarray) -> np.ndarray:
    """sign(M) packed into the device lhsT layout as fp8e4 bytes:
    wq[c2, half, kk*256 + ot*128 + o2] = sign(M[ot*128+o2, half*128+c2, kk]).
    +/-1.0 in fp8e4m3 is 0x38/0xB8 - exact."""
    s = np.where(M.reshape(O, C, K * K) > 0, np.uint8(0x38), np.uint8(0xB8))
    t = s.reshape(2, 128, 2, 128, K * K)  # [ot, o2, half, c2, kk]
    return np.ascontiguousarray(
        t.transpose(3, 2, 4, 0, 1).reshape(128, 2, K * K * O)
    )


def make_in_maps(x: np.ndarray, M: np.ndarray, Alpha: np.ndarray) -> list[dict]:
    x = np.ascontiguousarray(x, dtype=np.float32)
    wqa = pack_weights(np.asarray(M, dtype=np.float32))
    a = np.ascontiguousarray(Alpha, dtype=np.float32).reshape(O)
    return [
        {"x": x[i * NB : (i + 1) * NB], "wq": wqa, "alpha": a}
        for i in range(N_CORES)
    ]


def kernel(x: np.ndarray, M: np.ndarray, Alpha: np.ndarray) -> np.ndarray:
    """Full (unsharded) inputs in, full output out. Runs on 8 NeuronCores."""
    assert x.shape == (N_CORES * NB, C, H, W), x.shape
    nc = get_nc()
    in_maps = make_in_maps(x, M, Alpha)
    last_err = None
    for attempt in range(3):
        try:
            res = run_bass_kernel_spmd(nc, in_maps, list(range(N_CORES)))
            break
        except Exception as e:  # transient NRT/axon faults recover on retry
            last_err = e
            time.sleep(10 * (attempt + 1))
    else:
        raise last_err
    return np.concatenate([res.results[i]["out"] for i in range(N_CORES)], axis=0)


# revision 76
# speedup vs baseline: 1.0519x; 1.0519x over previous
"""Binarized 3x3 conv (BinarizeConv2dSDP) for one TRN2 chip (8 NeuronCores).

Reference computation:
    out = conv2d(sign(x), sign(M), stride=1, pad=1) * Alpha      (all fp32)
    x: (32, 256, 56, 56)   M: (256, 256, 3, 3)   Alpha: (256, 1, 1)

Strategy (per the data-parallel sharding hint):
  - Shard x over batch: 4 images per core; replicate the weights/Alpha on
    every core.
  - The weight tensor (0.004% of the FLOPs) is packed on the host into the
    fp8 +/-1 transposed-lhsT layout the PE consumes (sign() is exact in
    fp8, so this is pure input marshalling); each core DMAs the 0.59MB
    packed block once. The conv itself - all 59 GFLOP - runs on device.
  - On-core: binarize x to fp8 on the ACT engine, run the conv as 9
    shifted DoubleRow matmuls per 8-row strip (contraction = 256 channels:
    128 partitions x 2 pair-rows) accumulating in PSUM, scale by Alpha
    while evacuating PSUM, DMA out fp32.
  - Activations live in SBUF as zero-padded 58x58 images. Each matmul's
    moving AP is 2-level [8 rows x 56 cols] (row stride 58), so only the
    448 valid output columns of a strip are computed - no seam garbage.
  - Startup is latency-tuned: the packed weights land first on the sync
    HWDGE queue, image-0 arrives as 9/26/23-row chunks split across the
    sync+scalar queues, image-0's strips alternate ot blocks so early conv
    consumption matches the DMA arrival rate, act-tile zero fills run on
    GpSimd, and a short warmup burst ramps the HAM clock gate. Later
    image DMAs dispatch from conv-loop hooks (dispatching early would
    exhaust HWDGE ring credit and block the ACT engine's sign stream).
"""

import time

import numpy as np

import concourse.bacc as bacc
import concourse.bass as bass
import concourse.tile as tile
from concourse import mybir
from concourse.bass_utils import run_bass_kernel_spmd

F32 = mybir.dt.float32
BF16 = mybir.dt.bfloat16
FP8 = mybir.dt.float8e4
U8 = mybir.dt.uint8

# ---- problem geometry (hardcoded; kernel.py must be self-contained) ----
N_CORES = 8
NB = 4          # images per core (32 / 8)
C = 256         # in channels  (2 halves of 128 partitions)
O = 256         # out channels (2 tiles of 128 partitions)
H = W = 56
K = 3
PW = H + 2      # padded row width  (58)
NPIX = PW * PW  # padded image size (3364)
PH = 3376       # padded image allocation (multiple of 16)
RS = 8          # output rows per strip
NSTRIP = H // RS        # 7
NVAL = RS * W           # 448 psum columns per strip (valid only)

# image-0 chunking (input-row ranges) for early conv start
U_R0, U_NR = 0, 9       # ultra chunk -> early tile, strip 0
B_R0, B_NR = 7, 26      # covers padded rows 8..33 (strips 1-3, + s4 top)
C_R0, C_NR = 33, 23     # covers padded rows 34..56 (strips 4-6)
EROWS = U_NR + 1        # early tile padded rows 0..9
EPH = 592               # 10*58=580 rounded up to a multiple of 16

N_WARM = 70             # warmup matmuls (HAM clock ramp, fills PE to conv)
OUT_SPLIT = 12          # out-DMAs before this gidx go via GpSimd SWDGE


def build_nc() -> bass.Bass:
    """Build the SPMD Bass program for one core's shard."""
    nc = bacc.Bacc("TRN2")

    x = nc.declare_dram_parameter("x", [NB, C, H, W], F32, isOutput=False)
    # host-packed weights: wq[c2, half, kk*256 + ot*128 + o2]
    #   = sign(M[ot*128+o2, half*128+c2, kh, kw]) as fp8e4 bytes
    wq = nc.declare_dram_parameter("wq", [128, 2, K * K * O], U8, isOutput=False)
    alpha = nc.declare_dram_parameter("alpha", [O], F32, isOutput=False)
    out = nc.declare_dram_parameter("out", [NB, O, H, W], F32, isOutput=True)

    with tile.TileContext(nc) as tc:
        with (
            tc.tile_pool(name="consts", bufs=1) as consts,
            tc.tile_pool(name="xsrc", bufs=6) as xsrc_pool,
            tc.tile_pool(name="xsrc0", bufs=6) as xsrc0_pool,
            tc.tile_pool(name="osb", bufs=8) as osb_pool,
            tc.tile_pool(name="pmm", bufs=6, space="PSUM") as pmm_pool,
        ):
            # alpha: two tiny scattered DMAs on the GpSimd SWDGE ring
            alpha_sb = consts.tile([128, 2], F32)
            for ot, eng in ((0, nc.gpsimd), (1, nc.gpsimd)):
                eng.dma_start(
                    out=alpha_sb[:, ot : ot + 1],
                    in_=alpha.rearrange("(t o) -> t o", t=2)[ot].rearrange(
                        "(o u) -> o u", u=1
                    ),
                )

            # wz before any DVE dma dispatch: warmup matmuls gate on it
            wz = consts.tile([128, 256], BF16)
            nc.vector.memset(wz[:], 0)

            wbuf = consts.tile([128, 2, K * K * O], FP8)

            def x_chunk_dma(eng, n, half, r0, nr):
                xs = xsrc0_pool.tile([128, B_NR * W], F32)
                eng.dma_start(
                    out=xs[:, : nr * W],
                    in_=x[n, half * 128 : (half + 1) * 128, r0 : r0 + nr, :].rearrange(
                        "c h w -> c (h w)"
                    ),
                )
                return xs

            def x_img_dma(eng, n, half):
                xs = xsrc_pool.tile([128, H * W], F32)
                eng.dma_start(
                    out=xs[:],
                    in_=x[n, half * 128 : (half + 1) * 128].rearrange(
                        "c h w -> c (h w)"
                    ),
                )
                return xs

            # conv-start-critical chunks (xu, wq) ride the fast sync queue.
            # xu goes FIRST: it needs an ACT sign after landing while the
            # packed weights need no post-processing, so the last-arriving
            # transfer is the one without a processing tail. B/C go to the
            # scalar queue, which gets the DMA engines to itself once sync
            # quiets (~16us).
            # both xu halves in ONE dispatch: wq-h0's dispatch (and queue
            # entry) moves ~0.7us earlier on the sync engine
            xuc = xsrc0_pool.tile([128, 2, U_NR * W], F32)
            nc.sync.dma_start(
                out=xuc[:],
                in_=x[0, :, U_R0 : U_R0 + U_NR, :].rearrange(
                    "(a c) h w -> c a (h w)", a=2
                ),
            )
            xu = [xuc[:, h2] for h2 in range(2)]
            # wq split by c-half across both queues: halves the descriptor
            # rounds behind xu and overlaps the two transfers. On scalar,
            # wq-h1 is sandwiched between the B halves: B0 first for the
            # strip-1 deadline, wq-h1 next for an early conv start.
            nc.sync.dma_start(out=wbuf[:, 0].bitcast(U8), in_=wq[:, 0])
            nc.scalar.dma_start(out=wbuf[:, 1].bitcast(U8), in_=wq[:, 1])
            xb = [x_chunk_dma(nc.scalar, 0, h2, B_R0, B_NR) for h2 in range(2)]
            xc = [x_chunk_dma(nc.sync, 0, h2, C_R0, C_NR) for h2 in range(2)]

            # ---- PE warm-up: ramps the HAM clock gate (K=4/8 -> 8/8)
            # while the weight + image-0 DMAs land ----
            # bulk warmups bridge the PE to the typical conv-ready time
            # (~17.5us); a <=3.2us idle gap re-throttles HAM to half clock
            # for 10us, which costs far more than a slight overshoot. The
            # small trailing matmuls give the bridge fine granularity.
            pwarm = pmm_pool.tile([128, NVAL], F32, tag="pm")
            for _ in range(N_WARM):
                nc.tensor.matmul(
                    pwarm[:, :256], wz[:, :128], wz[:], start=True, stop=True
                )
            for _ in range(6):
                nc.tensor.matmul(
                    pwarm[:, :64], wz[:, :128], wz[:, :64], start=True, stop=True
                )

            # activation tiles: zero fill on GpSimd (keeps DVE/ACT free).
            # Image 0 + early tile + image 1 now; images 2-3 deferred.
            act = consts.tile([128, 2 * NB, PH], FP8)
            acte = consts.tile([128, 2, EPH], FP8)
            nc.gpsimd.memset(
                acte.rearrange("p a b -> p (a b)").bitcast(mybir.dt.uint32), 0
            )

            def act_memset(n):
                nc.gpsimd.memset(
                    act[:, 2 * n : 2 * n + 2, :]
                    .rearrange("p a b -> p (a b)")
                    .bitcast(mybir.dt.uint32),
                    0,
                )

            act_memset(0)
            act_memset(1)

            # ---- image-0 signs: ultra -> early tile, B/C -> main act ----
            for h2 in range(2):
                dst = acte[:, h2, : EROWS * PW].rearrange("p (h w) -> p h w", w=PW)[
                    :, 1 : 1 + U_NR, 1 : W + 1
                ]
                nc.scalar.sign(
                    dst, xu[h2][:, : U_NR * W].rearrange("p (h w) -> p h w", w=W)
                )

            # B in two row chunks, in DMA-arrival order, so strips 1-3
            # unblock as early as possible
            def b_sign(h2, r0, r1):
                dst = act[:, h2, :NPIX].rearrange("p (h w) -> p h w", w=PW)[
                    :, 1 + B_R0 + r0 : 1 + B_R0 + r1, 1 : W + 1
                ]
                nc.scalar.sign(
                    dst,
                    xb[h2][:, r0 * W : r1 * W].rearrange("p (h w) -> p h w", w=W),
                )

            b_sign(0, 0, 10)
            b_sign(1, 0, 10)
            b_sign(0, 10, B_NR)
            b_sign(1, 10, B_NR)
            for h2 in range(2):
                dst = act[:, h2, :NPIX].rearrange("p (h w) -> p h w", w=PW)[
                    :, 1 + C_R0 : 1 + C_R0 + C_NR, 1 : W + 1
                ]
                nc.scalar.sign(
                    dst, xc[h2][:, : C_NR * W].rearrange("p (h w) -> p h w", w=W)
                )

            def sign_img(n, half, xs):
                dst = act[:, 2 * n + half, :NPIX].rearrange(
                    "p (h w) -> p h w", w=PW
                )[:, 1 : H + 1, 1 : W + 1]
                nc.scalar.sign(dst, xs.rearrange("p (h w) -> p h w", w=W))

            # deferred work hooks: (gidx -> thunks) slipped into the conv
            # loop. Image DMAs dispatch here (ring credit is free by then);
            # their signs follow once the transfers have landed.
            ximg = {}

            def img_dma(eng, n, half):
                ximg[(n, half)] = x_img_dma(eng, n, half)

            def img_sign(n, half):
                sign_img(n, half, ximg[(n, half)])

            # image 1 follows image-0's chunks on the sync queue; images
            # 2-3 split across sync+scalar with wide deadline margins
            hooks = {
                4: [lambda: img_dma(nc.sync, 1, 0)],
                6: [lambda: img_dma(nc.sync, 1, 1)],
                8: [lambda: img_sign(1, 0), lambda: act_memset(2)],
                10: [lambda: img_sign(1, 1), lambda: img_dma(nc.sync, 2, 0)],
                12: [lambda: img_dma(nc.scalar, 2, 1)],
                14: [lambda: act_memset(3), lambda: img_dma(nc.sync, 3, 0)],
                16: [lambda: img_sign(2, 0), lambda: img_dma(nc.scalar, 3, 1)],
                17: [lambda: img_sign(2, 1)],
                20: [lambda: img_sign(3, 0)],
                21: [lambda: img_sign(3, 1)],
            }

            # ---- main conv loop ----
            def conv_strip(n, ot, s, gidx, r0=0, nr=RS, last_piece=False):
                pm = pmm_pool.tile([128, nr * W], F32)
                for kk in range(K * K):
                    kh, kw = divmod(kk, K)
                    base = (RS * s + r0 + kh) * PW + kw
                    lhsT2 = wbuf[:, :, kk * O + ot * 128 : kk * O + ot * 128 + 128]
                    src = acte if (n == 0 and s == 0) else act[:, 2 * n : 2 * n + 2]
                    # 2-level moving AP: nr rows x 56 valid cols
                    rhs4 = src[:, :, base : base + nr * PW].rearrange(
                        "p a (r w) -> p a r w", w=PW
                    )[:, :, :, :W]
                    nc.tensor.matmul(
                        pm[:],
                        lhsT2,
                        rhs4,
                        start=(kk == 0),
                        stop=(kk == K * K - 1),
                        perf_mode=mybir.MatmulPerfMode.DoubleRow,
                    )
                # evacuate, scaled by per-channel alpha
                osb = osb_pool.tile([128, nr * W], F32)
                nc.vector.tensor_scalar_mul(osb[:], pm[:], alpha_sb[:, ot : ot + 1])
                # early outs via GpSimd SWDGE (sync HWDGE is busy with
                # inputs); late outs via the then-idle sync ring; the final
                # strips' halves drain on parallel queues (only sync,
                # scalar and gpsimd can dispatch DMAs)
                if gidx < OUT_SPLIT:
                    eng = nc.gpsimd
                elif r0 > 0:
                    eng = nc.scalar
                else:
                    eng = nc.sync
                eng.dma_start(
                    out=out[
                        n,
                        ot * 128 : (ot + 1) * 128,
                        RS * s + r0 : RS * s + r0 + nr,
                        :,
                    ].rearrange("o h w -> o (h w)"),
                    in_=osb[:],
                )

            # image 0 alternates ot within each strip so early conv
            # consumption (2 groups per strip) tracks the chunk arrivals;
            # later images go ot-major (their data is fully resident).
            order = [(0, ot, s) for s in range(NSTRIP) for ot in range(2)]
            for n in range(1, NB):
                order += [(n, ot, s) for ot in range(2) for s in range(NSTRIP)]

            for gidx, (n, ot, s) in enumerate(order):
                if gidx == len(order) - 1:
                    # final strip in shrinking pieces (4,2,1,1 rows): each
                    # piece's evac+DMA overlaps the next piece's matmuls,
                    # and the very last transfer carries only one row
                    for r0, nr in ((0, 4), (4, 2), (6, 1), (7, 1)):
                        conv_strip(n, ot, s, gidx, r0, nr)
                elif gidx == len(order) - 2:
                    conv_strip(n, ot, s, gidx, 0, RS // 2)
                    conv_strip(n, ot, s, gidx, RS // 2, RS // 2)
                else:
                    conv_strip(n, ot, s, gidx)
                for h in hooks.get(gidx, ()):
                    h()
    nc.finalize()
    return nc


_NC_CACHE: dict[bool, bass.Bass] = {}


def get_nc(paired: bool = True) -> bass.Bass:
    if paired not in _NC_CACHE:
        _NC_CACHE[paired] = build_nc()
    return _NC_CACHE[paired]


def pack_weights(M: np.ndarray) -> np.ndarray:
    """sign(M) packed into the device lhsT layout as fp8e4 bytes:
    wq[c2, half, kk*256 + ot*128 + o2] = sign(M[ot*128+o2, half*128+c2, kk]).
    +/-1.0 in fp8e4m3 is 0x38/0xB8 - exact."""
    s = np.where(M.reshape(O, C, K * K) > 0, np.uint8(0x38), np.uint8(0xB8))
    t = s.reshape(2, 128, 2, 128, K * K)  # [ot, o2, half, c2, kk]
    return np.ascontiguousarray(
        t.transpose(3, 2, 4, 0, 1).reshape(128, 2, K * K * O)
    )


def make_in_maps(x: np.ndarray, M: np.ndarray, Alpha: np.ndarray) -> list[dict]:
    x = np.ascontiguousarray(x, dtype=np.float32)
    wqa = pack_weights(np.asarray(M, dtype=np.float32))
    a = np.ascontiguousarray(Alpha, dtype=np.float32).reshape(O)
    return [
        {"x": x[i * NB : (i + 1) * NB], "wq": wqa, "alpha": a}
        for i in range(N_CORES)
    ]


def kernel(x: np.ndarray, M: np.ndarray, Alpha: np.ndarray) -> np.ndarray:
    """Full (unsharded) inputs in, full output out. Runs on 8 NeuronCores."""
    assert x.shape == (N_CORES * NB, C, H, W), x.shape
    nc = get_nc()
    in_maps = make_in_maps(x, M, Alpha)
    last_err = None
    for attempt in range(3):
        try:
            res = run_bass_kernel_spmd(nc, in_maps, list(range(N_CORES)))
            break
        except Exception as e:  # transient NRT/axon faults recover on retry
            last_err = e
            time.sleep(10 * (attempt + 1))
    else:
        raise last_err
    return np.concatenate([res.results[i]["out"] for i in range(N_CORES)], axis=0)
